# revision 4
# baseline (speedup 1.0000x reference)
"""BiDirectionalMinGRU Trainium2 kernel (8 NeuronCores, batch-parallel).

Math: the reference computes, per direction,
    inp = x @ W_in + b_in                    (B,L,H)
    z   = sigmoid(inp @ Wz + bz)             (B,L,H)
    ht  = tanh(inp @ Wh + bh)
    h_t = (1-z_t) * h_{t-1} + z_t * ht_t     (scan over L)
    pred_t = h_bi_used(t) . Wg + bg
Both projections are linear in x, so they collapse into one (3 -> H)
affine map per gate, precomputed on the host in float64:
    z = sigmoid(x @ (W_in@Wz) + (b_in@Wz + bz)),  same for ht.

Device kernel per core (4 batch rows), per direction, in chunks of 512
timesteps (time laid out on the SBUF free dimension, hidden on
partitions):
  - TensorE: z_pre/ht_pre = W9^T @ x9, K=9 bf16 hi/lo-split operands
    (full fp32-grade precision at bf16 speed)
  - ScalarE: z = sigmoid(u+c), a = sigmoid(-u-c) = 1-z, ht = tanh(v+c)
  - VectorE: b = z*ht, then the hardware linear-recurrence instruction
    tensor_tensor_scan computes h[t] = a[t]*h[t-1] + b[t] along the
    free dim, chained across chunks via the carry column
  - TensorE: pred(1,512) += Wg_tile^T @ h_used   (float32r)
  - pred PSUM -> SBUF -> DRAM
The backward direction runs on host-pre-time-reversed x (identical
compute), and the host un-reverses, sums directions, and adds bg.

Known environment pitfalls baked in: fp32 matmuls with K=128 fail to
load (use bf16/f32r); f32r matmul operands must be *produced* as f32r
(scan writes f32r directly; weight copies via tensor_copy); memset
cannot write f32r; every dma_start AP must be 2-D.
"""

import sys

sys.path.insert(0, "/opt/trn_rl_repo")

import numpy as np
import ml_dtypes

B, L, H = 32, 4096, 256
N_CORES = 8
BL = B // N_CORES  # batch rows per core
CH = 512           # chunk length along L
NLB = L // CH

_compiled = None


def _build():
    global _compiled
    if _compiled is not None:
        return _compiled

    import concourse.tile as tile
    from concourse import bacc, mybir
    from contextlib import ExitStack

    f32 = mybir.dt.float32
    f32r = mybir.dt.float32r
    bf16 = mybir.dt.bfloat16
    AF = mybir.ActivationFunctionType
    OP = mybir.AluOpType

    nc = bacc.Bacc("TRN2", target_bir_lowering=False, debug=False,
                   num_devices=N_CORES)

    xt_d = nc.dram_tensor("xt", [2, BL, 9, L], bf16, kind="ExternalInput").ap()
    wa_d = nc.dram_tensor("wa", [9, 1024], bf16, kind="ExternalInput").ap()
    wb_d = nc.dram_tensor("wb", [128, 12], f32, kind="ExternalInput").ap()
    wg_d = nc.dram_tensor("wg", [128, 4], f32, kind="ExternalInput").ap()
    out_d = nc.dram_tensor("preds", [2 * BL, L], f32, kind="ExternalOutput").ap()

    with tile.TileContext(nc) as tc, ExitStack() as ctx:
        wpool = ctx.enter_context(tc.tile_pool(name="w", bufs=1))
        xpool = ctx.enter_context(tc.tile_pool(name="x", bufs=6))
        spool = ctx.enter_context(tc.tile_pool(name="s", bufs=3))
        hpool = ctx.enter_context(tc.tile_pool(name="h", bufs=36))
        opool = ctx.enter_context(tc.tile_pool(name="o", bufs=4))
        uvpool = ctx.enter_context(tc.tile_pool(name="uv", bufs=3, space="PSUM"))
        ppool = ctx.enter_context(tc.tile_pool(name="pp", bufs=2, space="PSUM"))

        wa_sb = wpool.tile([9, 1024], bf16)
        nc.sync.dma_start(wa_sb[:, :], wa_d[:, :])
        wb_sb = wpool.tile([128, 12], f32)
        nc.sync.dma_start(wb_sb[:, :], wb_d[:, :])
        wg_f = wpool.tile([128, 4], f32)
        nc.sync.dma_start(wg_f[:, :], wg_d[:, :])
        wg_sb = wpool.tile([128, 4], f32r)
        nc.vector.tensor_copy(wg_sb[:, :], wg_f[:, :])
        zcol = wpool.tile([128, 1], f32)
        nc.vector.memset(zcol[:, :], 0.0)

        prev_h = {}
        for lb in range(NLB):
            for d in range(2):
                for b in range(BL):
                    sl = slice(lb * CH, (lb + 1) * CH)
                    x_t = xpool.tile([9, CH], bf16)
                    nc.sync.dma_start(x_t[:, :], xt_d[d, b, :, sl])
                    pred_ps = ppool.tile([1, CH], f32)
                    for t in range(2):
                        u = uvpool.tile([128, CH], f32)
                        v = uvpool.tile([128, CH], f32)
                        c0 = d * 512 + t * 128
                        nc.tensor.matmul(u[:, :], wa_sb[:, c0:c0 + 128],
                                         x_t[:, :], start=True, stop=True)
                        nc.tensor.matmul(v[:, :], wa_sb[:, c0 + 256:c0 + 384],
                                         x_t[:, :], start=True, stop=True)
                        jb = d * 6 + t
                        zt = spool.tile([128, CH], f32)
                        nc.scalar.activation(zt[:, :], u[:, :], AF.Sigmoid,
                                             bias=wb_sb[:, jb:jb + 1])
                        at = spool.tile([128, CH], f32)
                        nc.scalar.activation(at[:, :], u[:, :], AF.Sigmoid,
                                             bias=wb_sb[:, jb + 2:jb + 3],
                                             scale=-1.0)
                        ht = spool.tile([128, CH], f32)
                        nc.scalar.activation(ht[:, :], v[:, :], AF.Tanh,
                                             bias=wb_sb[:, jb + 4:jb + 5])
                        bt = spool.tile([128, CH], f32)
                        nc.vector.tensor_tensor(bt[:, :], zt[:, :], ht[:, :],
                                                OP.mult)
                        h_t = hpool.tile([128, CH + 1], f32r)
                        if lb == 0:
                            nc.vector.tensor_copy(h_t[:, 0:1], zcol[:, :])
                        else:
                            nc.vector.tensor_copy(h_t[:, 0:1],
                                                  prev_h[(d, b, t)][:, CH:CH + 1])
                        nc.vector.tensor_tensor_scan(
                            h_t[:, 1:CH + 1], at[:, :], bt[:, :],
                            h_t[:, 0:1], OP.mult, OP.add)
                        prev_h[(d, b, t)] = h_t
                        jg = d * 2 + t
                        nc.tensor.matmul(pred_ps[:, :], wg_sb[:, jg:jg + 1],
                                         h_t[:, 0:CH],
                                         start=(t == 0), stop=(t == 1))
                    po = opool.tile([1, CH], f32)
                    nc.vector.tensor_copy(po[:, :], pred_ps[:, :])
                    j = d * BL + b
                    nc.sync.dma_start(out_d[j:j + 1, sl], po[0:1, :])

    nc.compile()
    _compiled = nc
    return nc


def _split_bf16(a32):
    hi = a32.astype(ml_dtypes.bfloat16)
    lo = (a32 - hi.astype(np.float32)).astype(ml_dtypes.bfloat16)
    return hi, lo


def _host_prep(x, Wf_in, bf_in, Wb_in, bb_in,
               Wz_f, bz_f, Wh_f, bh_f,
               Wz_b, bz_b, Wh_b, bh_b, Wg, bg):
    def col(Win, bin_, W, bw):
        A = (Win.astype(np.float64) @ W.astype(np.float64)).astype(np.float32)
        c = (bin_.astype(np.float64) @ W.astype(np.float64)
             + bw.astype(np.float64)).astype(np.float32)
        return A, c

    Azf, czf = col(Wf_in, bf_in, Wz_f, bz_f)
    Ahf, chf = col(Wf_in, bf_in, Wh_f, bh_f)
    Azb, czb = col(Wb_in, bb_in, Wz_b, bz_b)
    Ahb, chb = col(Wb_in, bb_in, Wh_b, bh_b)

    wa32 = np.concatenate([Azf, Ahf, Azb, Ahb], axis=1)  # (3, 1024) f32
    wa_hi, wa_lo = _split_bf16(wa32)
    wa = np.concatenate([wa_hi, wa_lo, wa_hi], axis=0)   # (9, 1024) bf16

    wb = np.empty((128, 12), np.float32)
    for d, (cz, ch) in enumerate([(czf, chf), (czb, chb)]):
        for t in range(2):
            seg = slice(t * 128, (t + 1) * 128)
            wb[:, d * 6 + t] = cz[seg]
            wb[:, d * 6 + 2 + t] = -cz[seg]
            wb[:, d * 6 + 4 + t] = ch[seg]

    wg = np.empty((128, 4), np.float32)
    for d in range(2):
        for t in range(2):
            wg[:, d * 2 + t] = Wg[d * 256 + t * 128: d * 256 + (t + 1) * 128, 0]

    in_maps = []
    for c in range(N_CORES):
        xs = np.ascontiguousarray(
            x[c * BL:(c + 1) * BL].astype(np.float32))   # (BL, L, 3)
        xt = np.empty((2, BL, 9, L), ml_dtypes.bfloat16)
        for d in range(2):
            xd = xs if d == 0 else xs[:, ::-1, :]
            x3 = xd.transpose(0, 2, 1)                   # (BL, 3, L) f32
            hi, lo = _split_bf16(x3)
            xt[d, :, 0:3] = hi
            xt[d, :, 3:6] = hi
            xt[d, :, 6:9] = lo
        in_maps.append({"xt": xt, "wa": wa, "wb": wb, "wg": wg})
    return in_maps, float(bg[0])


def _assemble(results, bg0):
    out = np.empty((B, L, 1), np.float32)
    for c in range(N_CORES):
        p = results[c]["preds"]  # (2*BL, L) f32
        pf = p[:BL]
        pb = p[BL:, ::-1]
        out[c * BL:(c + 1) * BL, :, 0] = pf + pb + bg0
    return out


def kernel(**inputs):
    from concourse.bass_utils import run_bass_kernel_spmd

    nc = _build()
    in_maps, bg0 = _host_prep(**inputs)
    res = run_bass_kernel_spmd(nc, in_maps, list(range(N_CORES)))
    return _assemble(res.results, bg0)


def benchmark(**inputs):
    """Run with NTFF tracing; returns (output, exec_time_ns)."""
    from concourse.bass_utils import run_bass_kernel_spmd

    nc = _build()
    in_maps, bg0 = _host_prep(**inputs)
    res = run_bass_kernel_spmd(nc, in_maps, list(range(N_CORES)), trace=True)
    return _assemble(res.results, bg0), res.exec_time_ns
